# revision 1
# baseline (speedup 1.0000x reference)
"""Hamming-similarity (BSC associative memory) kernel for 8 TRN2 NeuronCores.

reference: logit[b, c] = #matching bits between query[b] and am[c]
         = D - sum_d q - sum_d a + 2 * (q . a)
With bipolar x' = 2x - 1 in {-1, +1}:  (q' . a') = 2*logit - D, so
         logit = 0.5 * (q' @ a'^T) + D/2
One GEMM on +-1 data (exact in bf16) + scale/bias epilogue.

Sharding: data-parallel over the batch (4096 -> 512 per core), AM replicated.
Host side pre-transposes to [D, B] / [D, C] so the contraction dim lands on
SBUF partitions, pads D 10000 -> 10240 (80 chunks of 128), and casts the
bipolar values to bf16 (exact). Each core computes logit^T [100, 512]; the
host concatenates and transposes back.
"""

import numpy as np
import ml_dtypes

import concourse.bass as bass
import concourse.mybir as mybir
import concourse.tile as tile
from concourse import bacc
from concourse.bass_utils import run_bass_kernel_spmd

N_CORES = 8
BATCH = 4096
DIM = 10000
C = 100
B = BATCH // N_CORES  # 512 per core
P = 128
KC = 80               # contraction chunks of 128
D_PAD = KC * P        # 10240
GROUP = 16            # d-chunks per qT DMA (2 MB bf16)

_DT = mybir.dt.bfloat16
_NPDT = ml_dtypes.bfloat16

_CACHE: dict = {}


def _build():
    nc = bacc.Bacc("TRN2", target_bir_lowering=False, debug=False, num_devices=N_CORES)

    qT = nc.dram_tensor("qT", [D_PAD, B], _DT, kind="ExternalInput")
    amT = nc.dram_tensor("amT", [D_PAD, C], _DT, kind="ExternalInput")
    out = nc.dram_tensor("out", [C, B], mybir.dt.float32, kind="ExternalOutput")

    qT_r = qT.ap().rearrange("(o p) b -> p o b", p=P)    # [128, 80, 512]
    amT_r = amT.ap().rearrange("(o p) c -> p o c", p=P)  # [128, 80, 100]

    with tile.TileContext(nc) as tc:
        with (
            tc.tile_pool(name="am", bufs=1) as am_pool,
            tc.tile_pool(name="q", bufs=3) as q_pool,
            tc.tile_pool(name="ps", bufs=1, space="PSUM") as ps_pool,
            tc.tile_pool(name="ob", bufs=1) as ob_pool,
        ):
            am_sb = am_pool.tile([P, KC, C], _DT)
            nc.sync.dma_start(am_sb[:], amT_r[:])

            psum = ps_pool.tile([C, B], mybir.dt.float32)
            for g in range(KC // GROUP):
                q_sb = q_pool.tile([P, GROUP, B], _DT)
                nc.sync.dma_start(q_sb[:], qT_r[:, g * GROUP : (g + 1) * GROUP, :])
                for j in range(GROUP):
                    k = g * GROUP + j
                    nc.tensor.matmul(
                        psum[:],
                        am_sb[:, k, :],
                        q_sb[:, j, :],
                        start=(k == 0),
                        stop=(k == KC - 1),
                    )

            out_sb = ob_pool.tile([C, B], mybir.dt.float32)
            nc.scalar.activation(
                out_sb[:],
                psum[:],
                mybir.ActivationFunctionType.Copy,
                bias=float(DIM) / 2.0,
                scale=0.5,
            )
            nc.sync.dma_start(out.ap(), out_sb[:])

    nc.compile()
    return nc


def _get_nc():
    if "nc" not in _CACHE:
        _CACHE["nc"] = _build()
    return _CACHE["nc"]


def _prep_inputs(query: np.ndarray, am: np.ndarray):
    query = np.asarray(query, dtype=np.float32)
    am = np.asarray(am, dtype=np.float32)

    amT = np.zeros((D_PAD, C), dtype=_NPDT)
    amT[:DIM] = (2.0 * am - 1.0).T.astype(_NPDT)

    in_maps = []
    for i in range(N_CORES):
        q_i = query[i * B : (i + 1) * B]              # [512, 10000]
        qT_i = np.zeros((D_PAD, B), dtype=_NPDT)
        qT_i[:DIM] = (2.0 * q_i - 1.0).T.astype(_NPDT)
        in_maps.append({"qT": qT_i, "amT": amT})
    return in_maps


def _run(query: np.ndarray, am: np.ndarray, **kwargs):
    in_maps = _prep_inputs(query, am)
    res = run_bass_kernel_spmd(_get_nc(), in_maps, list(range(N_CORES)), **kwargs)
    logitT = np.concatenate(
        [res.results[i]["out"] for i in range(N_CORES)], axis=1
    )  # [100, 4096]
    return np.ascontiguousarray(logitT.T).astype(np.float32), res


def kernel(query: np.ndarray, am: np.ndarray) -> np.ndarray:
    out, _ = _run(query, am)
    return out


# revision 2
# speedup vs baseline: 1.2846x; 1.2846x over previous
"""Hamming-similarity (BSC associative memory) kernel for 8 TRN2 NeuronCores.

reference: logit[b, c] = #matching bits between query[b] and am[c]
         = D - sum_d q - sum_d a + 2 * (q . a)
With bipolar x' = 2x - 1 in {-1, +1}:  (q' . a') = 2*logit - D, so
         logit = 0.5 * (q' @ a'^T) + D/2
One GEMM on +-1 data (exact in fp8/bf16) + scale/bias epilogue.

Sharding: data-parallel over the batch (4096 -> 512 per core), AM replicated.
The host pre-bipolarizes, casts to fp8 (exact for +-1), pads D 10000 -> 10240
(80 chunks of 128), and pre-swizzles both operands into the exact SBUF layout
[128 partitions, chunk-major columns] so every DMA descriptor is a fat
contiguous run. Each core computes logit^T [100, 512]; the host concatenates
and transposes back.
"""

import numpy as np
import ml_dtypes

import concourse.bass as bass
import concourse.mybir as mybir
import concourse.tile as tile
from concourse import bacc
from concourse.bass_utils import run_bass_kernel_spmd

N_CORES = 8
BATCH = 4096
DIM = 10000
C = 100
B = BATCH // N_CORES  # 512 per core
P = 128
KC = 80               # contraction chunks of 128
D_PAD = KC * P        # 10240
GROUP = 8             # d-chunks per qT DMA (512 KB fp8, 4 KB/partition)

_DT = mybir.dt.float8e4
_NPDT = ml_dtypes.float8_e4m3

_CACHE: dict = {}


def _build():
    nc = bacc.Bacc("TRN2", target_bir_lowering=False, debug=False, num_devices=N_CORES)

    # Host supplies the SBUF-layout swizzle directly:
    #   qT_s[p, k*B + b]  = bipolar(query)[b_global, k*128 + p]
    #   amT_s[p, k*C + c] = bipolar(am)[c, k*128 + p]
    qT = nc.dram_tensor("qT", [P, KC * B], _DT, kind="ExternalInput")
    amT = nc.dram_tensor("amT", [P, KC * C], _DT, kind="ExternalInput")
    out = nc.dram_tensor("out", [C, B], mybir.dt.float32, kind="ExternalOutput")

    qT_r = qT.ap().rearrange("p (o b) -> p o b", b=B)    # [128, 80, 512]
    amT_r = amT.ap().rearrange("p (o c) -> p o c", c=C)  # [128, 80, 100]

    with tile.TileContext(nc) as tc:
        with (
            tc.tile_pool(name="am", bufs=1) as am_pool,
            tc.tile_pool(name="q", bufs=4) as q_pool,
            tc.tile_pool(name="ps", bufs=1, space="PSUM") as ps_pool,
            tc.tile_pool(name="ob", bufs=1) as ob_pool,
        ):
            am_sb = am_pool.tile([P, KC, C], _DT)
            nc.sync.dma_start(am_sb[:], amT_r[:])

            psum = ps_pool.tile([C, B], mybir.dt.float32)
            for g in range(KC // GROUP):
                q_sb = q_pool.tile([P, GROUP, B], _DT)
                nc.sync.dma_start(q_sb[:], qT_r[:, g * GROUP : (g + 1) * GROUP, :])
                for j in range(GROUP):
                    k = g * GROUP + j
                    nc.tensor.matmul(
                        psum[:],
                        am_sb[:, k, :],
                        q_sb[:, j, :],
                        start=(k == 0),
                        stop=(k == KC - 1),
                    )

            out_sb = ob_pool.tile([C, B], mybir.dt.float32)
            nc.scalar.activation(
                out_sb[:],
                psum[:],
                mybir.ActivationFunctionType.Copy,
                bias=float(DIM) / 2.0,
                scale=0.5,
            )
            nc.sync.dma_start(out.ap(), out_sb[:])

    nc.compile()
    return nc


def _get_nc():
    if "nc" not in _CACHE:
        _CACHE["nc"] = _build()
    return _CACHE["nc"]


def _swizzle(matT: np.ndarray, cols: int) -> np.ndarray:
    """[rows<=D_PAD, cols] bipolar f32 -> fp8 [128, KC*cols] chunk-major."""
    full = np.zeros((D_PAD, cols), dtype=_NPDT)
    full[: matT.shape[0]] = matT.astype(_NPDT)
    # [KC, 128, cols] -> [128, KC, cols] -> [128, KC*cols]
    return np.ascontiguousarray(
        full.reshape(KC, P, cols).transpose(1, 0, 2).reshape(P, KC * cols)
    )


def _prep_inputs(query: np.ndarray, am: np.ndarray):
    query = np.asarray(query, dtype=np.float32)
    am = np.asarray(am, dtype=np.float32)

    amT_s = _swizzle((2.0 * am - 1.0).T, C)

    in_maps = []
    for i in range(N_CORES):
        q_i = query[i * B : (i + 1) * B]  # [512, 10000]
        qT_s = _swizzle((2.0 * q_i - 1.0).T, B)
        in_maps.append({"qT": qT_s, "amT": amT_s})
    return in_maps


def _run(query: np.ndarray, am: np.ndarray, **kwargs):
    in_maps = _prep_inputs(query, am)
    res = run_bass_kernel_spmd(_get_nc(), in_maps, list(range(N_CORES)), **kwargs)
    logitT = np.concatenate(
        [res.results[i]["out"] for i in range(N_CORES)], axis=1
    )  # [100, 4096]
    return np.ascontiguousarray(logitT.T).astype(np.float32), res


def kernel(query: np.ndarray, am: np.ndarray) -> np.ndarray:
    out, _ = _run(query, am)
    return out


# revision 5
# speedup vs baseline: 1.8008x; 1.4019x over previous
"""Hamming-similarity (BSC associative memory) kernel for 8 TRN2 NeuronCores.

reference: logit[b, c] = #matching bits between query[b] and am[c]
         = D - sum_d q - sum_d a + 2 * (q . a)
With bipolar x' = 2x - 1 in {-1, +1}:  (q' . a') = 2*logit - D, so
         logit = 0.5 * (q' @ a'^T) + D/2
One GEMM on +-1 data (exact in fp8/bf16) + scale/bias epilogue.

Sharding: data-parallel over the batch (4096 -> 512 per core), AM replicated.
The host pre-bipolarizes, casts to fp8 (exact for +-1), pads D 10000 -> 10240
(80 chunks of 128), and pre-swizzles both operands into the exact SBUF layout
[128 partitions, chunk-major columns] so every DMA descriptor is a fat
contiguous run. Each core computes logit^T [100, 512]; the host concatenates
and transposes back.
"""

import numpy as np
import ml_dtypes

import concourse.bass as bass
import concourse.mybir as mybir
import concourse.tile as tile
from concourse import bacc
from concourse.bass_utils import run_bass_kernel_spmd

N_CORES = 8
BATCH = 4096
DIM = 10000
C = 100
C_PAD = 128           # class dim padded for DoubleRow AP alignment
B = BATCH // N_CORES  # 512 per core
P = 128
KC = 80               # contraction chunks of 128
D_PAD = KC * P        # 10240
GROUP = 16            # d-chunks per qT DMA (1 MB fp8, 8 KB/partition)

_DT = mybir.dt.float8e4
_NPDT = ml_dtypes.float8_e4m3

_CACHE: dict = {}


def _build():
    nc = bacc.Bacc("TRN2", target_bir_lowering=False, debug=False, num_devices=N_CORES)

    # Host supplies the SBUF-layout swizzle directly:
    #   qT_s[p, k*B + b]  = bipolar(query)[b_global, k*128 + p]
    #   amT_s[p, k*C_PAD + c] = bipolar(am)[c, k*128 + p]
    qT = nc.dram_tensor("qT", [P, KC * B], _DT, kind="ExternalInput")
    amT = nc.dram_tensor("amT", [P, KC * C_PAD], _DT, kind="ExternalInput")
    out = nc.dram_tensor("out", [C, B], mybir.dt.float32, kind="ExternalOutput")

    qT_r = qT.ap().rearrange("p (o b) -> p o b", b=B)        # [128, 80, 512]
    amT_r = amT.ap().rearrange("p (o c) -> p o c", c=C_PAD)  # [128, 80, 128]

    with tile.TileContext(nc) as tc:
        with (
            tc.tile_pool(name="am", bufs=1) as am_pool,
            tc.tile_pool(name="q", bufs=4) as q_pool,
            tc.tile_pool(name="ps", bufs=1, space="PSUM") as ps_pool,
            tc.tile_pool(name="ob", bufs=1) as ob_pool,
        ):
            am_sb = am_pool.tile([P, KC, C_PAD], _DT)
            nc.sync.dma_start(am_sb[:], amT_r[:])

            psum = ps_pool.tile([C_PAD, B], mybir.dt.float32)
            for g in range(KC // GROUP):
                q_sb = q_pool.tile([P, GROUP, B], _DT)
                nc.sync.dma_start(q_sb[:], qT_r[:, g * GROUP : (g + 1) * GROUP, :])
                for j in range(0, GROUP, 2):
                    k = g * GROUP + j
                    # fp8 DoubleRow: contract 2 chunks (256 rows) per matmul
                    nc.tensor.matmul(
                        psum[:],
                        am_sb[:, k : k + 2, :],
                        q_sb[:, j : j + 2, :],
                        start=(k == 0),
                        stop=(k == KC - 2),
                        perf_mode=mybir.MatmulPerfMode.DoubleRow,
                    )

            out_sb = ob_pool.tile([C, B], mybir.dt.float32)
            nc.vector.tensor_scalar(
                out_sb[:],
                psum[:C, :],
                0.5,
                float(DIM) / 2.0,
                mybir.AluOpType.mult,
                mybir.AluOpType.add,
            )
            nc.sync.dma_start(out.ap(), out_sb[:])

    nc.compile()
    return nc


def _get_nc():
    if "nc" not in _CACHE:
        _CACHE["nc"] = _build()
    return _CACHE["nc"]


def _swizzle(matT: np.ndarray, cols: int) -> np.ndarray:
    """[rows<=D_PAD, cols] bipolar f32 -> fp8 [128, KC*cols] chunk-major."""
    full = np.zeros((D_PAD, cols), dtype=_NPDT)
    full[: matT.shape[0]] = matT.astype(_NPDT)
    # [KC, 128, cols] -> [128, KC, cols] -> [128, KC*cols]
    return np.ascontiguousarray(
        full.reshape(KC, P, cols).transpose(1, 0, 2).reshape(P, KC * cols)
    )


def _prep_inputs(query: np.ndarray, am: np.ndarray):
    query = np.asarray(query, dtype=np.float32)
    am = np.asarray(am, dtype=np.float32)

    am_pad = np.zeros((C_PAD, DIM), dtype=np.float32)
    am_pad[:C] = 2.0 * am - 1.0
    amT_s = _swizzle(am_pad.T, C_PAD)

    in_maps = []
    for i in range(N_CORES):
        q_i = query[i * B : (i + 1) * B]  # [512, 10000]
        qT_s = _swizzle((2.0 * q_i - 1.0).T, B)
        in_maps.append({"qT": qT_s, "amT": amT_s})
    return in_maps


def _run(query: np.ndarray, am: np.ndarray, **kwargs):
    in_maps = _prep_inputs(query, am)
    res = run_bass_kernel_spmd(_get_nc(), in_maps, list(range(N_CORES)), **kwargs)
    logitT = np.concatenate(
        [res.results[i]["out"] for i in range(N_CORES)], axis=1
    )  # [100, 4096]
    return np.ascontiguousarray(logitT.T).astype(np.float32), res


def kernel(query: np.ndarray, am: np.ndarray) -> np.ndarray:
    out, _ = _run(query, am)
    return out
